# revision 32
# baseline (speedup 1.0000x reference)
"""Trainium2 Bass kernel for BitNet-style cross-attention (8 NeuronCores).

Strategy: pure data-parallel token sharding. b=2, n=2048 -> 4096 query-token
rows; each of the 8 cores owns 512 of them (cores 0-3 batch 0, 4-7 batch 1)
and computes its output slice fully independently (k/v for the core's batch
are recomputed per core).

All device tensors are feature-major ([dim, tokens]) so no on-chip transposes
are needed; the host supplies transposed views (pure layout transform).

Engine assignment (from HW trace analysis): TensorScalar ops with a runtime
per-partition pointer scalar run ~7ns/col on DVE and ~14.6ns/col on GpSimd
(pathological), while the Act engine applies pointer scales at ~0.9ns/col and
DVE immediates/tensor_tensor run at ~0.4-1ns/col. Act f32->int8 output
rounds to nearest-even (matches jnp.round).

Dataflow:
  - weight ternary: Act rounds w*(1/mean|w|) straight to int8; a single DVE
    immediate-clip widens to bf16 {-1,0,1}. mean|w| via Act Abs+accum_out.
  - activations quantize in T-major layout; the cross-partition absmax uses
    one gpsimd partition_all_reduce per sub-block (result replicated = free
    broadcast). The x path is dequantized on DVE; the ctx path stays integer
    (exact in bf16): its per-token scale inv_c is transposed to a per-128-
    block column via a tiny replicated-row matmul and applied per-partition:
    in the Exp scale (logits) and the Exp bias ln(inv_c) (v dequant), with
    the softmax denominator columns holding 1/(inv_c*mWv) to compensate.
  - emission order software-pipelines the act-quant sub-blocks (the
    DVE->gpsimd->DVE round trip would otherwise head-of-line-block the
    in-order DVE queue), and head-pair 0's attention runs fused with the
    v projection so the Act engine's exp stream starts ~30us earlier.
"""

import numpy as np

import concourse.bass as bass
import concourse.mybir as mybir
import concourse.tile as tile
from concourse import bacc, bass_isa
from concourse.bass_utils import run_bass_kernel_spmd

F32 = mybir.dt.float32
BF16 = mybir.dt.bfloat16
I8 = mybir.dt.int8
AX = mybir.AxisListType
OP = mybir.AluOpType
AF = mybir.ActivationFunctionType

P = 128

CFG_FULL = dict(DIM=1024, INNER=1024, H=16, D=64, NTOK=512, MCTX=2048)
N_CORES = 8
EPS = 1e-5


def build(cfg):
    DIM, INNER, H, D = cfg["DIM"], cfg["INNER"], cfg["H"], cfg["D"]
    NTOK, MCTX = cfg["NTOK"], cfg["MCTX"]
    KC = DIM // P          # input-dim chunks
    IC = INNER // P        # inner-dim chunks
    NKB = MCTX // P        # key blocks
    NTB = NTOK // P        # query-token 128-blocks
    CTB = MCTX // 512 if MCTX >= 512 else 1   # ctx 512-col blocks for k proj
    CW = min(512, MCTX)    # k-proj moving width
    NH = INNER // 512 if INNER >= 512 else 1  # inner 512-halves
    IW = min(512, INNER)
    SUB = min(256, NTOK)   # act-quant token sub-block
    VW = D + 1             # v columns per head incl denominator col

    nc = bacc.Bacc("TRN2", target_bir_lowering=False, debug=False,
                   num_devices=N_CORES)

    xT = nc.dram_tensor("xT", [DIM, NTOK], F32, kind="ExternalInput")
    cT = nc.dram_tensor("cT", [DIM, MCTX], F32, kind="ExternalInput")
    wT = {}
    for w in ("wq", "wk", "wv", "wo"):
        wT[w] = nc.dram_tensor(w + "T", [DIM, INNER], F32, kind="ExternalInput")
    y_out = nc.dram_tensor("y", [NTOK, DIM], F32, kind="ExternalOutput")

    from contextlib import ExitStack
    with tile.TileContext(nc) as tc, ExitStack() as ctx:
        pp = ctx.enter_context(tc.tile_pool(name="persist", bufs=1))
        smp = ctx.enter_context(tc.tile_pool(name="small", bufs=2))
        wsp = ctx.enter_context(tc.tile_pool(name="wstage", bufs=2))
        wbp = ctx.enter_context(tc.tile_pool(name="wbpool", bufs=2))
        ep = ctx.enter_context(tc.tile_pool(name="etile", bufs=4))
        ps_proj = ctx.enter_context(tc.tile_pool(name="ps_proj", bufs=2,
                                                 space="PSUM"))
        ps_sc = ctx.enter_context(tc.tile_pool(name="ps_sc", bufs=2,
                                               space="PSUM"))
        ps_o = ctx.enter_context(tc.tile_pool(name="ps_o", bufs=2,
                                              space="PSUM"))

        # ---- persistent SBUF tensors (live across phases) ----------------
        qb = pp.tile([P, IC * NTOK], BF16, tag="qb")      # q deq, T-major
        kb = pp.tile([P, IC * MCTX], BF16, tag="kb")      # k int, T-major
        vb = pp.tile([P, NKB * H * VW], BF16, tag="vb")   # v int + denom col
        invT = pp.tile([P, NKB], F32, tag="invT")         # ctx tok scale cols
        qkinv = pp.tile([P, NKB], F32, tag="qkinv")       # exp scale cols
        lninvT = pp.tile([P, NKB], F32, tag="lninvT")     # exp bias cols
        ones128 = pp.tile([P, 1], F32, tag="ones128")
        nc.vector.memset(ones128[:], 1.0 / 128.0)
        vb3 = vb[:].rearrange("p (k h w) -> p k h w", h=H, w=VW)
        cd3 = None  # set after cdT exists

        wmean, wqs = {}, {}

        def quant_weight_mean(w):
            wpart = smp.tile([P, KC], F32, tag="wpart")
            for c in range(KC):
                s = wsp.tile([P, INNER], F32, tag="wst")
                nc.sync.dma_start(out=s[:], in_=wT[w].ap()[c * P:(c + 1) * P, :])
                wsc = wsp.tile([P, INNER], F32, tag="wsc")
                nc.scalar.activation(wsc[:], s[:], AF.Abs,
                                     accum_out=wpart[:, c:c + 1])
            wsum = smp.tile([P, 1], F32, tag="wsum")
            nc.vector.tensor_reduce(wsum[:], wpart[:], axis=AX.X, op=OP.add)
            wrep = smp.tile([P, 1], F32, tag="wrep")
            nc.gpsimd.partition_all_reduce(wrep[:], wsum[:], channels=P,
                                           reduce_op=bass_isa.ReduceOp.add)
            mean = smp.tile([P, 1], F32, tag="wmean_" + w, name="mean_" + w)
            nc.vector.tensor_scalar(mean[:], wrep[:], 1.0 / (DIM * INNER),
                                    EPS, OP.mult, OP.max)
            qs = smp.tile([P, 1], F32, tag="wqs_" + w, name="qs_" + w)
            nc.vector.reciprocal(qs[:], mean[:])
            wmean[w], wqs[w] = mean, qs

        def quant_weight_ternary(w):
            # Act rounds w*qs straight to int8 (round-half-even, matches
            # jnp.round); |w*qs| < 127 so no saturation happens here, and the
            # ternary clip to [-1,1] rides the DVE widening cast
            qs = wqs[w]
            wbt = wbp.tile([P, KC * INNER], BF16, tag="wb", name="wb_" + w)
            for c in range(KC):
                s = wsp.tile([P, INNER], F32, tag="wst")
                nc.sync.dma_start(out=s[:], in_=wT[w].ap()[c * P:(c + 1) * P, :])
                t8 = wsp.tile([P, INNER], I8, tag="wt8")
                nc.scalar.mul(t8[:], s[:], qs[:])
                nc.vector.tensor_scalar(wbt[:, c * INNER:(c + 1) * INNER],
                                        t8[:], 1.0, -1.0, OP.min, OP.max)
            return wbt

        # ---- activation quantization (T-major), 2-stage pipelined --------
        def aq_front(asp, srcT, ncols, c0):
            stage = asp.tile([P, KC, SUB], F32, tag="astage")
            nc.sync.dma_start(
                out=stage[:],
                in_=srcT.ap()[:, c0:c0 + SUB].rearrange("(c p) s -> p c s",
                                                        p=P))
            pam = asp.tile([P, SUB], F32, tag="apam")
            nc.vector.tensor_reduce(
                pam[:], stage[:].rearrange("p c s -> p s c"),
                axis=AX.X, op=OP.max, apply_absolute_value=True)
            arep = asp.tile([P, SUB], F32, tag="arep")
            nc.gpsimd.partition_all_reduce(
                arep[:], pam[:], channels=P,
                reduce_op=bass_isa.ReduceOp.absmax)
            return stage, arep

        def aq_finish(asp, stage, arep, dstT, ncols, c0, deq):
            inv = asp.tile([P, SUB], F32, tag="ainv")
            nc.vector.tensor_scalar(inv[:], arep[:], EPS, 1.0 / 127.0,
                                    OP.max, OP.mult)
            qsc = asp.tile([P, SUB], F32, tag="aqsc")
            nc.vector.reciprocal_approx_fast(qsc[:], inv[:])
            if deq:
                for c in range(KC):
                    i8 = asp.tile([P, SUB], I8, tag="ai8")
                    nc.vector.tensor_tensor(i8[:], stage[:, c, :], qsc[:],
                                            op=OP.mult)
                    nc.vector.tensor_tensor(
                        dstT[:, c * ncols + c0:c * ncols + c0 + SUB],
                        i8[:], inv[:], op=OP.mult)
            else:
                i8b = asp.tile([P, KC, SUB], I8, tag="ai8b")
                for c in range(KC):
                    nc.vector.tensor_tensor(i8b[:, c, :], stage[:, c, :],
                                            qsc[:], op=OP.mult)
                d3 = dstT[:].rearrange("p (c n) -> p c n", c=KC)
                nc.scalar.copy(d3[:, :, c0:c0 + SUB], i8b[:])
                # inv is replicated across partitions; out[t,0] =
                # sum_p inv[p,t]/128 = inv[t] puts token t's scale on
                # partition t for each 128-token block
                for jb in range(SUB // P):
                    kbk = (c0 + jb * P) // P
                    psi = ps_o.tile([P, NTOK], F32, tag="po",
                                    name=f"psi{kbk}")
                    nc.tensor.matmul(psi[:, 0:1],
                                     inv[:, jb * P:(jb + 1) * P],
                                     ones128[:], start=True, stop=True)
                    nc.vector.tensor_copy(invT[:, kbk:kbk + 1], psi[:, 0:1])

        # ---- attention building blocks -----------------------------------
        def hp_geom(hp):
            hA, hB = 2 * hp, 2 * hp + 1
            return [(hA, (hA * D) // P, (hA * D) % P),
                    (hB, (hB * D) // P, (hB * D) % P)]

        def attn_kbk(hp, kbk, po):
            ss = ps_sc.tile([P, 2, NTOK], F32, tag="ss", name="ss")
            for j, (h, ich, ph) in enumerate(hp_geom(hp)):
                nc.tensor.matmul(
                    ss[:, j, :],
                    kb[ph:ph + D,
                       ich * MCTX + kbk * P: ich * MCTX + (kbk + 1) * P],
                    qb[ph:ph + D, ich * NTOK:(ich + 1) * NTOK],
                    start=True, stop=True)
            et = ep.tile([P, 2, NTOK], BF16, tag="et")
            nc.scalar.activation(et[:], ss[:], AF.Exp,
                                 scale=qkinv[:, kbk:kbk + 1],
                                 bias=lninvT[:, kbk:kbk + 1])
            for j, (h, ich, ph) in enumerate(hp_geom(hp)):
                nc.tensor.matmul(
                    po[j][0:VW, :],
                    vb3[:, kbk, h, :],
                    et[:, j, :],
                    start=(kbk == 0), stop=(kbk == NKB - 1))

        def attn_norm(hp, po, op_pool, otT, ot3, omx, omn):
            for j, (h, ich, ph) in enumerate(hp_geom(hp)):
                # reciprocal_approx_fast mis-handles inputs at a nonzero
                # partition offset: stage the denominator row (psum row D)
                # to a partition-0 tile first
                dn = op_pool.tile([1, NTOK], F32, tag="dn", bufs=1)
                nc.vector.tensor_copy(dn[:], po[j][D:D + 1, :])
                rd = op_pool.tile([1, NTOK], F32, tag="rd", bufs=2)
                nc.vector.reciprocal_approx_fast(rd[:], dn[:])
                rb = op_pool.tile([D, NTOK], F32, tag="rb", bufs=2)
                nc.gpsimd.partition_broadcast(rb[:], rd[:])
                nc.vector.tensor_tensor(
                    otT[ph:ph + D, ich * NTOK:(ich + 1) * NTOK],
                    po[j][0:D, :], rb[:], op=OP.mult)
            # head-pair hp fills otT chunk hp: fold it into the running
            # per-token max/min while later heads are still computing
            if hp == 0:
                nc.vector.tensor_copy(omx[:], ot3[:, 0, :])
                nc.vector.tensor_copy(omn[:], ot3[:, 0, :])
            else:
                nc.vector.tensor_tensor(omx[:], omx[:], ot3[:, hp, :],
                                        op=OP.max)
                nc.vector.tensor_tensor(omn[:], omn[:], ot3[:, hp, :],
                                        op=OP.min)

        po0 = None
        with ExitStack() as phase12:
            adp = phase12.enter_context(tc.tile_pool(name="adpool", bufs=1))
            asp = phase12.enter_context(tc.tile_pool(name="astage", bufs=2))
            xdT = adp.tile([P, KC * NTOK], BF16, tag="xdT")
            cdT = adp.tile([P, KC * MCTX], BF16, tag="cdT")

            # x quant (pipelined within the call), wq quant, q projection
            f0 = aq_front(asp, xT, NTOK, 0)
            f1 = aq_front(asp, xT, NTOK, SUB)
            aq_finish(asp, *f0, xdT, NTOK, 0, True)
            quant_weight_mean("wq")
            aq_finish(asp, *f1, xdT, NTOK, SUB, True)
            wqb = quant_weight_ternary("wq")
            for ic in range(IC):
                ps = ps_proj.tile([P, NTOK], F32, tag="pp", name="psq")
                for c in range(KC):
                    nc.tensor.matmul(
                        ps[:],
                        wqb[:, c * INNER + ic * P: c * INNER + (ic + 1) * P],
                        xdT[:, c * NTOK:(c + 1) * NTOK],
                        start=(c == 0), stop=(c == KC - 1))
                nc.vector.tensor_copy(qb[:, ic * NTOK:(ic + 1) * NTOK], ps[:])

            quant_weight_mean("wk")
            qkmul = smp.tile([P, 1], F32, tag="qkmul")
            nc.vector.tensor_tensor(qkmul[:], wmean["wq"][:], wmean["wk"][:],
                                    op=OP.mult)
            qksc = smp.tile([P, 1], F32, tag="qksc")
            nc.vector.tensor_scalar(qksc[:], qkmul[:], 1.0 / np.sqrt(D), None,
                                    OP.mult)
            wkb = quant_weight_ternary("wk")
            quant_weight_mean("wv")
            quant_weight_mean("wo")

            # ctx quant (pipelined across all 8 sub-blocks) interleaved with
            # per-512-block k projection
            pend = None
            for s in range(MCTX // SUB):
                front = aq_front(asp, cT, MCTX, s * SUB)
                if pend is not None:
                    aq_finish(asp, *pend)
                pend = (front[0], front[1], cdT, MCTX, s * SUB, False)
                if s % (CW // SUB) == (CW // SUB - 1) and s >= CW // SUB:
                    tb = s // (CW // SUB) - 1
                    for ic in range(IC):
                        ps = ps_proj.tile([P, CW], F32, tag="pp", name="psk")
                        for c in range(KC):
                            nc.tensor.matmul(
                                ps[:],
                                wkb[:, c * INNER + ic * P:
                                    c * INNER + (ic + 1) * P],
                                cdT[:, c * MCTX + tb * CW:
                                    c * MCTX + (tb + 1) * CW],
                                start=(c == 0), stop=(c == KC - 1))
                        nc.vector.tensor_copy(
                            kb[:, ic * MCTX + tb * CW:
                               ic * MCTX + (tb + 1) * CW],
                            ps[:])
            aq_finish(asp, *pend)
            tb = CTB - 1
            for ic in range(IC):
                ps = ps_proj.tile([P, CW], F32, tag="pp", name="psk")
                for c in range(KC):
                    nc.tensor.matmul(
                        ps[:],
                        wkb[:, c * INNER + ic * P: c * INNER + (ic + 1) * P],
                        cdT[:, c * MCTX + tb * CW: c * MCTX + (tb + 1) * CW],
                        start=(c == 0), stop=(c == KC - 1))
                nc.vector.tensor_copy(
                    kb[:, ic * MCTX + tb * CW: ic * MCTX + (tb + 1) * CW],
                    ps[:])

            # 5. exp scale/bias columns + denominator columns
            # v stays integer-valued in vb; its per-ctx-token dequant scale
            # inv_c rides in the Exp bias (exp(s+ln(inv)) = inv*exp(s)), and
            # the denominator column compensates with 1/(inv_c*mWv) so the
            # softmax reciprocal yields normalized, mWv-scaled output.
            wvb = quant_weight_ternary("wv")
            rmv = smp.tile([P, 1], F32, tag="rmv")
            nc.vector.reciprocal(rmv[:], wmean["wv"][:])
            nc.scalar.activation(lninvT[:], invT[:], AF.Ln)
            nc.scalar.mul(qkinv[:], invT[:], qksc[:])
            rinvT = smp.tile([P, NKB], F32, tag="rinvT", bufs=1)
            nc.vector.reciprocal_approx_fast(rinvT[:], invT[:])
            rmvT = smp.tile([P, NKB], F32, tag="rmvT", bufs=1)
            nc.vector.tensor_tensor(rmvT[:], rinvT[:],
                                    rmv[:].broadcast_to([P, NKB]), op=OP.mult)
            for kbk in range(NKB):
                nc.vector.tensor_copy(
                    vb3[:, kbk, :, D],
                    rmvT[:, kbk:kbk + 1].broadcast_to([P, H]))

            # 6. v projection fused with head-pair 0's attention: the Act
            # engine's exp stream starts while PE still projects v
            po0 = [ps_o.tile([P, NTOK], F32, tag="po", name=f"po0_{j}")
                   for j in range(2)]
            for kbk in range(NKB):
                for ih in range(NH):
                    ps = ps_proj.tile([P, IW], F32, tag="pp", name="psv")
                    for c in range(KC):
                        nc.tensor.matmul(
                            ps[:],
                            cdT[:, c * MCTX + kbk * P: c * MCTX + (kbk + 1) * P],
                            wvb[:, c * INNER + ih * IW: c * INNER + (ih + 1) * IW],
                            start=(c == 0), stop=(c == KC - 1))
                    hph = IW // D
                    nc.vector.tensor_copy(
                        vb3[:, kbk, ih * hph:(ih + 1) * hph, 0:D],
                        ps[:].rearrange("p (h d) -> p h d", d=D))
                attn_kbk(0, kbk, po0)

            # 7. wo ternary: overlaps the attention phase
            wob = quant_weight_ternary("wo")

        # ---- attention (head pairs 1..7) ---------------------------------
        op_pool = ctx.enter_context(tc.tile_pool(name="opool", bufs=1))
        otT = op_pool.tile([P, IC * NTOK], F32, tag="otT")
        oqdT = op_pool.tile([P, IC * NTOK], BF16, tag="oqdT")
        ot3 = otT[:].rearrange("p (c t) -> p c t", c=IC)
        omx = op_pool.tile([P, NTOK], F32, tag="omx", bufs=1)
        omn = op_pool.tile([P, NTOK], F32, tag="omn", bufs=1)
        attn_norm(0, po0, op_pool, otT, ot3, omx, omn)
        for hp in range(1, H // 2):
            popool, potag = (ps_o, "po") if hp % 2 == 0 else (ps_proj, "pp")
            po = [popool.tile([P, NTOK], F32, tag=potag, name=f"po{hp}_{j}")
                  for j in range(2)]
            for kbk in range(NKB):
                attn_kbk(hp, kbk, po)
            attn_norm(hp, po, op_pool, otT, ot3, omx, omn)

        # ---- attn-out quantization + output projection -------------------
        with tc.tile_pool(name="oq", bufs=2) as oqp, \
                tc.tile_pool(name="ysb", bufs=2) as yp:
            oamax = op_pool.tile([P, NTOK], F32, tag="oamax", bufs=1)
            nc.vector.tensor_scalar(oamax[:], omn[:], -1.0, None, OP.mult)
            nc.vector.tensor_tensor(oamax[:], oamax[:], omx[:], op=OP.max)
            oarep = oqp.tile([P, NTOK], F32, tag="oarep")
            nc.gpsimd.partition_all_reduce(
                oarep[:], oamax[:], channels=P,
                reduce_op=bass_isa.ReduceOp.absmax)
            oinv = op_pool.tile([P, NTOK], F32, tag="oinv", bufs=1)
            nc.vector.tensor_scalar(oinv[:], oarep[:], EPS, 1.0 / 127.0,
                                    OP.max, OP.mult)
            oqsc = op_pool.tile([P, NTOK], F32, tag="oqsc", bufs=1)
            nc.vector.reciprocal_approx_fast(oqsc[:], oinv[:])
            for c in range(IC):
                i8 = oqp.tile([P, NTOK], I8, tag="oi8")
                nc.vector.tensor_tensor(i8[:], ot3[:, c, :], oqsc[:], op=OP.mult)
                nc.vector.tensor_tensor(oqdT[:, c * NTOK:(c + 1) * NTOK],
                                        i8[:], oinv[:], op=OP.mult)

            for tb in range(NTB):
                for oh in range(DIM // IW):
                    ps = ps_proj.tile([P, IW], F32, tag="pp", name="psy")
                    for c in range(IC):
                        nc.tensor.matmul(
                            ps[:],
                            oqdT[:, c * NTOK + tb * P: c * NTOK + (tb + 1) * P],
                            wob[:, c * INNER + oh * IW: c * INNER + (oh + 1) * IW],
                            start=(c == 0), stop=(c == IC - 1))
                    ysb = yp.tile([P, IW], F32, tag="ysb")
                    nc.scalar.mul(ysb[:], ps[:], wmean["wo"][:])
                    nc.sync.dma_start(
                        out=y_out.ap()[tb * P:(tb + 1) * P,
                                       oh * IW:(oh + 1) * IW],
                        in_=ysb[:])
    nc.compile()
    return nc


_CACHE = {}


def _get_nc(key, cfg):
    if key not in _CACHE:
        _CACHE[key] = build(cfg)
    return _CACHE[key]


def _shard(x, context, wq, wk, wv, wo, NTOK):
    b = x.shape[0]
    wmaps = {w + "T": np.ascontiguousarray(a.T)
             for w, a in (("wq", wq), ("wk", wk), ("wv", wv), ("wo", wo))}
    cores_per_b = N_CORES // b
    in_maps = []
    for core in range(N_CORES):
        bi = core // cores_per_b
        t0 = (core % cores_per_b) * NTOK
        in_maps.append(dict(
            xT=np.ascontiguousarray(x[bi, t0:t0 + NTOK, :].T),
            cT=np.ascontiguousarray(context[bi].T),
            **wmaps))
    return in_maps


def _assemble(results, b, n, dim, NTOK):
    out = np.empty((b, n, dim), dtype=np.float32)
    cores_per_b = N_CORES // b
    for core in range(N_CORES):
        bi = core // cores_per_b
        t0 = (core % cores_per_b) * NTOK
        out[bi, t0:t0 + NTOK, :] = results[core]["y"]
    return out


def run(x, context, wq, wk, wv, wo, trace=False):
    cfg = CFG_FULL
    b, n, dim = x.shape
    NTOK = cfg["NTOK"]
    nc = _get_nc("full", cfg)
    in_maps = _shard(x, context, wq, wk, wv, wo, NTOK)
    res = run_bass_kernel_spmd(nc, in_maps, list(range(N_CORES)), trace=trace)
    return _assemble(res.results, b, n, dim, NTOK), res


def kernel(x, context, wq, wk, wv, wo):
    return run(x, context, wq, wk, wv, wo, trace=False)[0]


if __name__ == "__main__":
    ins = {k: np.random.randn(*s).astype(np.float32) * (0.02 if k[0] == 'w' else 1.0)
           for k, s in [("x", (2, 2048, 1024)), ("context", (2, 2048, 1024)),
                        ("wq", (1024, 1024)), ("wk", (1024, 1024)),
                        ("wv", (1024, 1024)), ("wo", (1024, 1024))]}
    y = kernel(**ins)
    print("kernel output", y.shape, y.dtype, np.abs(y).max())


# revision 34
# speedup vs baseline: 1.0359x; 1.0359x over previous
"""Trainium2 Bass kernel for BitNet-style cross-attention (8 NeuronCores).

Strategy: pure data-parallel token sharding. b=2, n=2048 -> 4096 query-token
rows; each of the 8 cores owns 512 of them (cores 0-3 batch 0, 4-7 batch 1)
and computes its output slice fully independently (k/v for the core's batch
are recomputed per core).

All device tensors are feature-major ([dim, tokens]) so no on-chip transposes
are needed; the host supplies transposed views (pure layout transform).

Engine assignment (from HW trace analysis): TensorScalar ops with a runtime
per-partition pointer scalar run ~7ns/col on DVE and ~14.6ns/col on GpSimd
(pathological), while the Act engine applies pointer scales at ~0.9ns/col and
DVE immediates/tensor_tensor run at ~0.4-1ns/col. Act f32->int8 output
rounds to nearest-even (matches jnp.round).

Dataflow:
  - weight ternary: Act rounds w*(1/mean|w|) straight to int8; a single DVE
    immediate-clip widens to bf16 {-1,0,1}. mean|w| via Act Abs+accum_out.
  - activations quantize in T-major layout; the cross-partition absmax uses
    one gpsimd partition_all_reduce per sub-block (result replicated = free
    broadcast). The x path is dequantized on DVE; the ctx path stays integer
    (exact in bf16): its per-token scale inv_c is transposed to a per-128-
    block column via a tiny replicated-row matmul and applied per-partition:
    in the Exp scale (logits) and the Exp bias ln(inv_c) (v dequant), with
    the softmax denominator columns holding 1/(inv_c*mWv) to compensate.
  - emission order software-pipelines the act-quant sub-blocks (the
    DVE->gpsimd->DVE round trip would otherwise head-of-line-block the
    in-order DVE queue), and head-pair 0's attention runs fused with the
    v projection so the Act engine's exp stream starts ~30us earlier.
"""

import numpy as np

import concourse.bass as bass
import concourse.mybir as mybir
import concourse.tile as tile
from concourse import bacc, bass_isa
from concourse.bass_utils import run_bass_kernel_spmd

F32 = mybir.dt.float32
BF16 = mybir.dt.bfloat16
I8 = mybir.dt.int8
AX = mybir.AxisListType
OP = mybir.AluOpType
AF = mybir.ActivationFunctionType

P = 128

CFG_FULL = dict(DIM=1024, INNER=1024, H=16, D=64, NTOK=512, MCTX=2048)
N_CORES = 8
EPS = 1e-5


def build(cfg):
    DIM, INNER, H, D = cfg["DIM"], cfg["INNER"], cfg["H"], cfg["D"]
    NTOK, MCTX = cfg["NTOK"], cfg["MCTX"]
    KC = DIM // P          # input-dim chunks
    IC = INNER // P        # inner-dim chunks
    NKB = MCTX // P        # key blocks
    NTB = NTOK // P        # query-token 128-blocks
    CTB = MCTX // 512 if MCTX >= 512 else 1   # ctx 512-col blocks for k proj
    CW = min(512, MCTX)    # k-proj moving width
    NH = INNER // 512 if INNER >= 512 else 1  # inner 512-halves
    IW = min(512, INNER)
    SUB = min(256, NTOK)   # act-quant token sub-block
    VW = D + 1             # v columns per head incl denominator col

    nc = bacc.Bacc("TRN2", target_bir_lowering=False, debug=False,
                   num_devices=N_CORES)

    xT = nc.dram_tensor("xT", [DIM, NTOK], F32, kind="ExternalInput")
    cT = nc.dram_tensor("cT", [DIM, MCTX], F32, kind="ExternalInput")
    wT = {}
    for w in ("wq", "wk", "wv", "wo"):
        wT[w] = nc.dram_tensor(w + "T", [DIM, INNER], F32, kind="ExternalInput")
    y_out = nc.dram_tensor("y", [NTOK, DIM], F32, kind="ExternalOutput")

    from contextlib import ExitStack
    with tile.TileContext(nc) as tc, ExitStack() as ctx:
        pp = ctx.enter_context(tc.tile_pool(name="persist", bufs=1))
        smp = ctx.enter_context(tc.tile_pool(name="small", bufs=2))
        wsp = ctx.enter_context(tc.tile_pool(name="wstage", bufs=2))
        wbp = ctx.enter_context(tc.tile_pool(name="wbpool", bufs=2))
        ep = ctx.enter_context(tc.tile_pool(name="etile", bufs=4))
        ps_proj = ctx.enter_context(tc.tile_pool(name="ps_proj", bufs=2,
                                                 space="PSUM"))
        ps_sc = ctx.enter_context(tc.tile_pool(name="ps_sc", bufs=2,
                                               space="PSUM"))
        ps_o = ctx.enter_context(tc.tile_pool(name="ps_o", bufs=2,
                                              space="PSUM"))

        # ---- persistent SBUF tensors (live across phases) ----------------
        qb = pp.tile([P, IC * NTOK], BF16, tag="qb")      # q deq, T-major
        kb = pp.tile([P, IC * MCTX], BF16, tag="kb")      # k int, T-major
        vb = pp.tile([P, NKB * H * VW], BF16, tag="vb")   # v int + denom col
        invT = pp.tile([P, NKB], F32, tag="invT")         # ctx tok scale cols
        qkinv = pp.tile([P, NKB], F32, tag="qkinv")       # exp scale cols
        lninvT = pp.tile([P, NKB], F32, tag="lninvT")     # exp bias cols
        ones128 = pp.tile([P, 1], F32, tag="ones128")
        nc.vector.memset(ones128[:], 1.0 / 128.0)
        vb3 = vb[:].rearrange("p (k h w) -> p k h w", h=H, w=VW)
        cd3 = None  # set after cdT exists

        wmean, wqs = {}, {}

        def quant_weight_mean(w):
            wpart = smp.tile([P, KC], F32, tag="wpart")
            for c in range(KC):
                s = wsp.tile([P, INNER], F32, tag="wst")
                nc.sync.dma_start(out=s[:], in_=wT[w].ap()[c * P:(c + 1) * P, :])
                wsc = wsp.tile([P, INNER], F32, tag="wsc")
                nc.scalar.activation(wsc[:], s[:], AF.Abs,
                                     accum_out=wpart[:, c:c + 1])
            wsum = smp.tile([P, 1], F32, tag="wsum")
            nc.vector.tensor_reduce(wsum[:], wpart[:], axis=AX.X, op=OP.add)
            wrep = smp.tile([P, 1], F32, tag="wrep")
            nc.gpsimd.partition_all_reduce(wrep[:], wsum[:], channels=P,
                                           reduce_op=bass_isa.ReduceOp.add)
            mean = smp.tile([P, 1], F32, tag="wmean_" + w, name="mean_" + w)
            nc.vector.tensor_scalar(mean[:], wrep[:], 1.0 / (DIM * INNER),
                                    EPS, OP.mult, OP.max)
            qs = smp.tile([P, 1], F32, tag="wqs_" + w, name="qs_" + w)
            nc.vector.reciprocal(qs[:], mean[:])
            wmean[w], wqs[w] = mean, qs

        def quant_weight_ternary(w):
            # Act rounds w*qs straight to int8 (round-half-even, matches
            # jnp.round); |w*qs| < 127 so no saturation happens here, and the
            # ternary clip to [-1,1] rides the DVE widening cast
            qs = wqs[w]
            wbt = wbp.tile([P, KC * INNER], BF16, tag="wb", name="wb_" + w)
            for c in range(KC):
                s = wsp.tile([P, INNER], F32, tag="wst")
                nc.sync.dma_start(out=s[:], in_=wT[w].ap()[c * P:(c + 1) * P, :])
                t8 = wsp.tile([P, INNER], I8, tag="wt8")
                nc.scalar.mul(t8[:], s[:], qs[:])
                nc.vector.tensor_scalar(wbt[:, c * INNER:(c + 1) * INNER],
                                        t8[:], 1.0, -1.0, OP.min, OP.max)
            return wbt

        # ---- activation quantization (T-major), 2-stage pipelined --------
        def aq_front(asp, srcT, ncols, c0):
            stage = asp.tile([P, KC, SUB], F32, tag="astage")
            nc.sync.dma_start(
                out=stage[:],
                in_=srcT.ap()[:, c0:c0 + SUB].rearrange("(c p) s -> p c s",
                                                        p=P))
            pam = asp.tile([P, SUB], F32, tag="apam")
            nc.vector.tensor_reduce(
                pam[:], stage[:].rearrange("p c s -> p s c"),
                axis=AX.X, op=OP.max, apply_absolute_value=True)
            arep = asp.tile([P, SUB], F32, tag="arep")
            nc.gpsimd.partition_all_reduce(
                arep[:], pam[:], channels=P,
                reduce_op=bass_isa.ReduceOp.absmax)
            return stage, arep

        def aq_finish(asp, stage, arep, dstT, ncols, c0, deq):
            inv = asp.tile([P, SUB], F32, tag="ainv")
            nc.vector.tensor_scalar(inv[:], arep[:], EPS, 1.0 / 127.0,
                                    OP.max, OP.mult)
            qsc = asp.tile([P, SUB], F32, tag="aqsc")
            nc.vector.reciprocal_approx_fast(qsc[:], inv[:])
            if deq:
                for c in range(KC):
                    i8 = asp.tile([P, SUB], I8, tag="ai8")
                    nc.vector.tensor_tensor(i8[:], stage[:, c, :], qsc[:],
                                            op=OP.mult)
                    nc.vector.tensor_tensor(
                        dstT[:, c * ncols + c0:c * ncols + c0 + SUB],
                        i8[:], inv[:], op=OP.mult)
            else:
                i8b = asp.tile([P, KC, SUB], I8, tag="ai8b")
                for c in range(KC):
                    nc.vector.tensor_tensor(i8b[:, c, :], stage[:, c, :],
                                            qsc[:], op=OP.mult)
                d3 = dstT[:].rearrange("p (c n) -> p c n", c=KC)
                nc.scalar.copy(d3[:, :, c0:c0 + SUB], i8b[:])
                # inv is replicated across partitions; out[t,0] =
                # sum_p inv[p,t]/128 = inv[t] puts token t's scale on
                # partition t for each 128-token block
                for jb in range(SUB // P):
                    kbk = (c0 + jb * P) // P
                    psi = ps_o.tile([P, NTOK], F32, tag="po",
                                    name=f"psi{kbk}")
                    nc.tensor.matmul(psi[:, 0:1],
                                     inv[:, jb * P:(jb + 1) * P],
                                     ones128[:], start=True, stop=True)
                    nc.vector.tensor_copy(invT[:, kbk:kbk + 1], psi[:, 0:1])

        # ---- attention building blocks -----------------------------------
        def hp_geom(hp):
            hA, hB = 2 * hp, 2 * hp + 1
            return [(hA, (hA * D) // P, (hA * D) % P),
                    (hB, (hB * D) // P, (hB * D) % P)]

        def attn_kbk(hp, kbk, po):
            ss = ps_sc.tile([P, 2, NTOK], F32, tag="ss", name="ss")
            for j, (h, ich, ph) in enumerate(hp_geom(hp)):
                nc.tensor.matmul(
                    ss[:, j, :],
                    kb[ph:ph + D,
                       ich * MCTX + kbk * P: ich * MCTX + (kbk + 1) * P],
                    qb[ph:ph + D, ich * NTOK:(ich + 1) * NTOK],
                    start=True, stop=True)
            et = ep.tile([P, 2, NTOK], BF16, tag="et")
            nc.scalar.activation(et[:], ss[:], AF.Exp,
                                 scale=qkinv[:, kbk:kbk + 1],
                                 bias=lninvT[:, kbk:kbk + 1])
            for j, (h, ich, ph) in enumerate(hp_geom(hp)):
                nc.tensor.matmul(
                    po[j][0:VW, :],
                    vb3[:, kbk, h, :],
                    et[:, j, :],
                    start=(kbk == 0), stop=(kbk == NKB - 1))

        def attn_norm(hp, po, op_pool, otT, ot3, omx, omn):
            for j, (h, ich, ph) in enumerate(hp_geom(hp)):
                # reciprocal_approx_fast mis-handles inputs at a nonzero
                # partition offset: stage the denominator row (psum row D)
                # to a partition-0 tile first
                dn = op_pool.tile([1, NTOK], F32, tag="dn", bufs=1)
                nc.vector.tensor_copy(dn[:], po[j][D:D + 1, :])
                rd = op_pool.tile([1, NTOK], F32, tag="rd", bufs=2)
                nc.vector.reciprocal_approx_fast(rd[:], dn[:])
                rb = op_pool.tile([D, NTOK], F32, tag="rb", bufs=2)
                nc.gpsimd.partition_broadcast(rb[:], rd[:])
                nc.vector.tensor_tensor(
                    otT[ph:ph + D, ich * NTOK:(ich + 1) * NTOK],
                    po[j][0:D, :], rb[:], op=OP.mult)
            # head-pair hp fills otT chunk hp: fold it into the running
            # per-token max/min while later heads are still computing
            if hp == 0:
                nc.vector.tensor_copy(omx[:], ot3[:, 0, :])
                nc.vector.tensor_copy(omn[:], ot3[:, 0, :])
            else:
                nc.vector.tensor_tensor(omx[:], omx[:], ot3[:, hp, :],
                                        op=OP.max)
                nc.vector.tensor_tensor(omn[:], omn[:], ot3[:, hp, :],
                                        op=OP.min)

        po0 = None
        with ExitStack() as phase12:
            adp = phase12.enter_context(tc.tile_pool(name="adpool", bufs=1))
            asp = phase12.enter_context(tc.tile_pool(name="astage", bufs=2))
            xdT = adp.tile([P, KC * NTOK], BF16, tag="xdT")
            cdT = adp.tile([P, KC * MCTX], BF16, tag="cdT")

            # x quant (pipelined within the call), wq quant, q projection
            f0 = aq_front(asp, xT, NTOK, 0)
            f1 = aq_front(asp, xT, NTOK, SUB)
            aq_finish(asp, *f0, xdT, NTOK, 0, True)
            quant_weight_mean("wq")
            aq_finish(asp, *f1, xdT, NTOK, SUB, True)
            wqb = quant_weight_ternary("wq")
            for ic in range(IC):
                ps = ps_proj.tile([P, NTOK], F32, tag="pp", name="psq")
                for c in range(KC):
                    nc.tensor.matmul(
                        ps[:],
                        wqb[:, c * INNER + ic * P: c * INNER + (ic + 1) * P],
                        xdT[:, c * NTOK:(c + 1) * NTOK],
                        start=(c == 0), stop=(c == KC - 1))
                nc.vector.tensor_copy(qb[:, ic * NTOK:(ic + 1) * NTOK], ps[:])

            quant_weight_mean("wk")
            qkmul = smp.tile([P, 1], F32, tag="qkmul")
            nc.vector.tensor_tensor(qkmul[:], wmean["wq"][:], wmean["wk"][:],
                                    op=OP.mult)
            qksc = smp.tile([P, 1], F32, tag="qksc")
            nc.vector.tensor_scalar(qksc[:], qkmul[:], 1.0 / np.sqrt(D), None,
                                    OP.mult)
            wkb = quant_weight_ternary("wk")

            # ctx quant (pipelined across all 8 sub-blocks) interleaved with
            # per-512-block k projection
            pend = None
            for s in range(MCTX // SUB):
                front = aq_front(asp, cT, MCTX, s * SUB)
                if pend is not None:
                    aq_finish(asp, *pend)
                pend = (front[0], front[1], cdT, MCTX, s * SUB, False)
                if s % (CW // SUB) == (CW // SUB - 1) and s >= CW // SUB:
                    tb = s // (CW // SUB) - 1
                    for ic in range(IC):
                        ps = ps_proj.tile([P, CW], F32, tag="pp", name="psk")
                        for c in range(KC):
                            nc.tensor.matmul(
                                ps[:],
                                wkb[:, c * INNER + ic * P:
                                    c * INNER + (ic + 1) * P],
                                cdT[:, c * MCTX + tb * CW:
                                    c * MCTX + (tb + 1) * CW],
                                start=(c == 0), stop=(c == KC - 1))
                        nc.vector.tensor_copy(
                            kb[:, ic * MCTX + tb * CW:
                               ic * MCTX + (tb + 1) * CW],
                            ps[:])
            aq_finish(asp, *pend)
            quant_weight_mean("wv")
            quant_weight_mean("wo")
            tb = CTB - 1
            for ic in range(IC):
                ps = ps_proj.tile([P, CW], F32, tag="pp", name="psk")
                for c in range(KC):
                    nc.tensor.matmul(
                        ps[:],
                        wkb[:, c * INNER + ic * P: c * INNER + (ic + 1) * P],
                        cdT[:, c * MCTX + tb * CW: c * MCTX + (tb + 1) * CW],
                        start=(c == 0), stop=(c == KC - 1))
                nc.vector.tensor_copy(
                    kb[:, ic * MCTX + tb * CW: ic * MCTX + (tb + 1) * CW],
                    ps[:])

            # 5. exp scale/bias columns + denominator columns
            # v stays integer-valued in vb; its per-ctx-token dequant scale
            # inv_c rides in the Exp bias (exp(s+ln(inv)) = inv*exp(s)), and
            # the denominator column compensates with 1/(inv_c*mWv) so the
            # softmax reciprocal yields normalized, mWv-scaled output.
            wvb = quant_weight_ternary("wv")
            rmv = smp.tile([P, 1], F32, tag="rmv")
            nc.vector.reciprocal(rmv[:], wmean["wv"][:])
            nc.scalar.activation(lninvT[:], invT[:], AF.Ln)
            nc.scalar.mul(qkinv[:], invT[:], qksc[:])
            rinvT = smp.tile([P, NKB], F32, tag="rinvT", bufs=1)
            nc.vector.reciprocal_approx_fast(rinvT[:], invT[:])
            rmvT = smp.tile([P, NKB], F32, tag="rmvT", bufs=1)
            nc.vector.tensor_tensor(rmvT[:], rinvT[:],
                                    rmv[:].broadcast_to([P, NKB]), op=OP.mult)
            for kbk in range(NKB):
                nc.vector.tensor_copy(
                    vb3[:, kbk, :, D],
                    rmvT[:, kbk:kbk + 1].broadcast_to([P, H]))

            # 6. v projection fused with head-pair 0's attention: the Act
            # engine's exp stream starts while PE still projects v
            po0 = [ps_o.tile([P, NTOK], F32, tag="po", name=f"po0_{j}")
                   for j in range(2)]
            for kbk in range(NKB):
                for ih in range(NH):
                    ps = ps_proj.tile([P, IW], F32, tag="pp", name="psv")
                    for c in range(KC):
                        nc.tensor.matmul(
                            ps[:],
                            cdT[:, c * MCTX + kbk * P: c * MCTX + (kbk + 1) * P],
                            wvb[:, c * INNER + ih * IW: c * INNER + (ih + 1) * IW],
                            start=(c == 0), stop=(c == KC - 1))
                    hph = IW // D
                    nc.vector.tensor_copy(
                        vb3[:, kbk, ih * hph:(ih + 1) * hph, 0:D],
                        ps[:].rearrange("p (h d) -> p h d", d=D))
                attn_kbk(0, kbk, po0)

            # 7. wo ternary: overlaps the attention phase
            wob = quant_weight_ternary("wo")

        # ---- attention (head pairs 1..7) ---------------------------------
        op_pool = ctx.enter_context(tc.tile_pool(name="opool", bufs=1))
        otT = op_pool.tile([P, IC * NTOK], F32, tag="otT")
        oqdT = op_pool.tile([P, IC * NTOK], BF16, tag="oqdT")
        ot3 = otT[:].rearrange("p (c t) -> p c t", c=IC)
        omx = op_pool.tile([P, NTOK], F32, tag="omx", bufs=1)
        omn = op_pool.tile([P, NTOK], F32, tag="omn", bufs=1)
        attn_norm(0, po0, op_pool, otT, ot3, omx, omn)
        for hp in range(1, H // 2):
            popool, potag = (ps_o, "po") if hp % 2 == 0 else (ps_proj, "pp")
            po = [popool.tile([P, NTOK], F32, tag=potag, name=f"po{hp}_{j}")
                  for j in range(2)]
            for kbk in range(NKB):
                attn_kbk(hp, kbk, po)
            attn_norm(hp, po, op_pool, otT, ot3, omx, omn)

        # ---- attn-out quantization + output projection -------------------
        with tc.tile_pool(name="oq", bufs=2) as oqp, \
                tc.tile_pool(name="ysb", bufs=2) as yp:
            oamax = op_pool.tile([P, NTOK], F32, tag="oamax", bufs=1)
            nc.vector.tensor_scalar(oamax[:], omn[:], -1.0, None, OP.mult)
            nc.vector.tensor_tensor(oamax[:], oamax[:], omx[:], op=OP.max)
            oarep = oqp.tile([P, NTOK], F32, tag="oarep")
            nc.gpsimd.partition_all_reduce(
                oarep[:], oamax[:], channels=P,
                reduce_op=bass_isa.ReduceOp.absmax)
            oinv = op_pool.tile([P, NTOK], F32, tag="oinv", bufs=1)
            nc.vector.tensor_scalar(oinv[:], oarep[:], EPS, 1.0 / 127.0,
                                    OP.max, OP.mult)
            oqsc = op_pool.tile([P, NTOK], F32, tag="oqsc", bufs=1)
            nc.vector.reciprocal_approx_fast(oqsc[:], oinv[:])
            for c in range(IC):
                i8 = oqp.tile([P, NTOK], I8, tag="oi8")
                nc.vector.tensor_tensor(i8[:], ot3[:, c, :], oqsc[:], op=OP.mult)
                nc.vector.tensor_tensor(oqdT[:, c * NTOK:(c + 1) * NTOK],
                                        i8[:], oinv[:], op=OP.mult)

            for tb in range(NTB):
                for oh in range(DIM // IW):
                    ps = ps_proj.tile([P, IW], F32, tag="pp", name="psy")
                    for c in range(IC):
                        nc.tensor.matmul(
                            ps[:],
                            oqdT[:, c * NTOK + tb * P: c * NTOK + (tb + 1) * P],
                            wob[:, c * INNER + oh * IW: c * INNER + (oh + 1) * IW],
                            start=(c == 0), stop=(c == IC - 1))
                    ysb = yp.tile([P, IW], F32, tag="ysb")
                    nc.scalar.mul(ysb[:], ps[:], wmean["wo"][:])
                    nc.sync.dma_start(
                        out=y_out.ap()[tb * P:(tb + 1) * P,
                                       oh * IW:(oh + 1) * IW],
                        in_=ysb[:])
    nc.compile()
    return nc


_CACHE = {}


def _get_nc(key, cfg):
    if key not in _CACHE:
        _CACHE[key] = build(cfg)
    return _CACHE[key]


def _shard(x, context, wq, wk, wv, wo, NTOK):
    b = x.shape[0]
    wmaps = {w + "T": np.ascontiguousarray(a.T)
             for w, a in (("wq", wq), ("wk", wk), ("wv", wv), ("wo", wo))}
    cores_per_b = N_CORES // b
    in_maps = []
    for core in range(N_CORES):
        bi = core // cores_per_b
        t0 = (core % cores_per_b) * NTOK
        in_maps.append(dict(
            xT=np.ascontiguousarray(x[bi, t0:t0 + NTOK, :].T),
            cT=np.ascontiguousarray(context[bi].T),
            **wmaps))
    return in_maps


def _assemble(results, b, n, dim, NTOK):
    out = np.empty((b, n, dim), dtype=np.float32)
    cores_per_b = N_CORES // b
    for core in range(N_CORES):
        bi = core // cores_per_b
        t0 = (core % cores_per_b) * NTOK
        out[bi, t0:t0 + NTOK, :] = results[core]["y"]
    return out


def run(x, context, wq, wk, wv, wo, trace=False):
    cfg = CFG_FULL
    b, n, dim = x.shape
    NTOK = cfg["NTOK"]
    nc = _get_nc("full", cfg)
    in_maps = _shard(x, context, wq, wk, wv, wo, NTOK)
    res = run_bass_kernel_spmd(nc, in_maps, list(range(N_CORES)), trace=trace)
    return _assemble(res.results, b, n, dim, NTOK), res


def kernel(x, context, wq, wk, wv, wo):
    return run(x, context, wq, wk, wv, wo, trace=False)[0]


if __name__ == "__main__":
    ins = {k: np.random.randn(*s).astype(np.float32) * (0.02 if k[0] == 'w' else 1.0)
           for k, s in [("x", (2, 2048, 1024)), ("context", (2, 2048, 1024)),
                        ("wq", (1024, 1024)), ("wk", (1024, 1024)),
                        ("wv", (1024, 1024)), ("wo", (1024, 1024))]}
    y = kernel(**ins)
    print("kernel output", y.shape, y.dtype, np.abs(y).max())
